# revision 13
# baseline (speedup 1.0000x reference)
"""Trainium2 Bass kernel for ternary-weight linear (plinear STE forward).

Reference math:
    y = x @ ((w_pos > 0) - (w_neg > 0)).T      # [8192, 4096]

Algebraic fold: the two binarized matmuls collapse into ONE matmul with a
ternary {-1,0,1} weight matrix, halving PE work. Ternary values are exact in
bf16, so the matmul runs at bf16 rate (2x fp32); only x is quantized to bf16.

Sharding (8 cores): 2 token-shards x 4 out-feature shards.
Per core: x_shard [4096, 4096] (bf16, staged transposed so K=in_features lands
on SBUF partitions), w slices [4096, 1024] (bf16, staged transposed),
binarize+subtract on device (DVE), then a K=4096 accumulated matmul with
x tiles stationary and ternary weights moving. Output [4096, 1024] fp32.
"""

import numpy as np
import ml_dtypes

P = 128
N_TOK, IN_F, OUT_F = 8192, 4096, 4096
TA, OB = 2, 4                 # token shards x out shards = 8 cores
T_S = N_TOK // TA             # 4096 tokens per shard
O_S = OUT_F // OB             # 1024 out features per shard
K_SUB = IN_F // P             # 32 k-subtiles
T_TILE = 256                  # tokens per streamed x tile
N_TT = T_S // T_TILE          # 16
N_FREE = 512                  # matmul moving free dim (one PSUM bank of fp32)

_CACHE = {}


def _build(repeats=1):
    key = ("nc", repeats)
    if key in _CACHE:
        return _CACHE[key]
    import concourse.bacc as bacc
    import concourse.mybir as mybir
    import concourse.tile as tile
    from concourse.bass import ds

    nc = bacc.Bacc("TRN2", target_bir_lowering=False, debug=False)
    # x pre-tiled on host: [tt, ki, ko, t] so each tile DMA is one
    # contiguous 16KB line per partition (no 512B scatter).
    xP = nc.dram_tensor("xP", (N_TT, P, K_SUB, T_TILE), mybir.dt.bfloat16,
                        kind="ExternalInput")
    wpT = nc.dram_tensor("wpT", (IN_F, O_S), mybir.dt.bfloat16, kind="ExternalInput")
    wnT = nc.dram_tensor("wnT", (IN_F, O_S), mybir.dt.bfloat16, kind="ExternalInput")
    y = nc.dram_tensor("y", (T_S, O_S), mybir.dt.float32, kind="ExternalOutput")

    wpT_r = wpT[:].rearrange("(ko ki) o -> ki ko o", ki=P)   # [128, 32, 1024]
    wnT_r = wnT[:].rearrange("(ko ki) o -> ki ko o", ki=P)
    y_r = y[:].rearrange("(to ti) o -> ti to o", ti=P)       # [128, 32, 1024]

    with tile.TileContext(nc) as tc:
        with (
            tc.tile_pool(name="tern", bufs=1) as tern_pool,
            tc.tile_pool(name="wstage", bufs=10) as wstage,
            tc.tile_pool(name="btmp", bufs=4) as btmp,
            tc.tile_pool(name="xp", bufs=3) as xp,
            tc.tile_pool(name="outp", bufs=3) as outp,
            tc.tile_pool(name="psum", bufs=4, space="PSUM") as psum_pool,
        ):
            for _rep in range(repeats):
                # ---- Phase A: ternary weights, K-major, SBUF-resident ----
                ternT = tern_pool.tile([P, K_SUB, O_S], mybir.dt.bfloat16)
                for k in range(K_SUB):
                    wp_t = wstage.tile([P, O_S], mybir.dt.bfloat16, tag="w")
                    wn_t = wstage.tile([P, O_S], mybir.dt.bfloat16, tag="w")
                    nc.sync.dma_start(wp_t[:], wpT_r[:, k, :])
                    nc.scalar.dma_start(wn_t[:], wnT_r[:, k, :])
                    bn = btmp.tile([P, O_S], mybir.dt.bfloat16, tag="b")
                    nc.vector.tensor_scalar(
                        bn[:], wn_t[:], 0.0, None, mybir.AluOpType.is_gt
                    )
                    # ternT = (wp > 0) - (wn > 0)
                    nc.vector.scalar_tensor_tensor(
                        ternT[:, k, :], wp_t[:], 0.0, bn[:],
                        mybir.AluOpType.is_gt, mybir.AluOpType.subtract,
                    )

                # ---- Phase B: y[t, o] = sum_k xT[k, t] * ternT[k, o] ----
                for tt in range(N_TT):
                    x_t = xp.tile([P, K_SUB, T_TILE], mybir.dt.bfloat16)
                    for kc in range(0, K_SUB, 8):
                        nc.gpsimd.dma_start(
                            x_t[:, kc:kc + 8, :], xP[tt, :, kc:kc + 8, :])
                    for m in range(T_TILE // P):
                        ps = psum_pool.tile([P, O_S], mybir.dt.float32)
                        for k in range(K_SUB):
                            for ob2 in range(O_S // N_FREE):
                                nc.tensor.matmul(
                                    ps[:, ob2 * N_FREE:(ob2 + 1) * N_FREE],
                                    x_t[:, k, m * P:(m + 1) * P],
                                    ternT[:, k, ob2 * N_FREE:(ob2 + 1) * N_FREE],
                                    start=(k == 0),
                                    stop=(k == K_SUB - 1),
                                )
                        o_t = outp.tile([P, O_S], mybir.dt.float32)
                        nc.vector.tensor_copy(o_t[:], ps[:])
                        nc.gpsimd.dma_start(
                            y_r[:, tt * (T_TILE // P) + m, :], o_t[:])
    nc.compile()
    _CACHE[key] = nc
    return nc


def _shard_inputs(x, w_pos, w_neg):
    bf16 = ml_dtypes.bfloat16
    xb = x.astype(bf16)                               # [N_TOK, IN_F]
    wpT = np.ascontiguousarray(w_pos.astype(bf16).T)  # [IN_F, OUT_F]
    wnT = np.ascontiguousarray(w_neg.astype(bf16).T)
    in_maps = []
    for c in range(TA * OB):
        ta, ob = divmod(c, OB)
        xs = xb[ta * T_S:(ta + 1) * T_S]              # [T_S, IN_F]
        # [tt, t, ko, ki] -> [tt, ki, ko, t]
        xp = np.ascontiguousarray(
            xs.reshape(N_TT, T_TILE, K_SUB, P).transpose(0, 3, 2, 1))
        in_maps.append({
            "xP": xp,
            "wpT": np.ascontiguousarray(wpT[:, ob * O_S:(ob + 1) * O_S]),
            "wnT": np.ascontiguousarray(wnT[:, ob * O_S:(ob + 1) * O_S]),
        })
    return in_maps


def run(x, w_pos, w_neg, trace=False):
    """Returns (y_full, BassKernelResults)."""
    from concourse import bass_utils

    nc = _build()
    in_maps = _shard_inputs(x, w_pos, w_neg)
    res = bass_utils.run_bass_kernel_spmd(
        nc, in_maps, core_ids=list(range(TA * OB)), trace=trace
    )
    y_full = np.empty((N_TOK, OUT_F), np.float32)
    for c in range(TA * OB):
        ta, ob = divmod(c, OB)
        y_full[ta * T_S:(ta + 1) * T_S, ob * O_S:(ob + 1) * O_S] = res.results[c]["y"]
    return y_full, res


def kernel(x, w_pos, w_neg):
    y, _ = run(x, w_pos, w_neg, trace=False)
    return y


# revision 14
# speedup vs baseline: 1.2559x; 1.2559x over previous
"""Trainium2 Bass kernel for ternary-weight linear (plinear STE forward).

Reference math:
    y = x @ ((w_pos > 0) - (w_neg > 0)).T      # [8192, 4096]

Algebraic fold: the two binarized matmuls collapse into ONE matmul with a
ternary {-1,0,1} weight matrix, halving PE work. Ternary values are exact in
bf16, so the matmul runs at bf16 rate (2x fp32); only x is quantized to bf16.

Sharding (8 cores): TA token-shards x OB out-feature shards.
Per core: x pre-tiled on host (so K=in_features lands on SBUF partitions and
every DMA is >=4KB-contiguous per partition), w slices staged transposed
(bf16 cast only -- sign is preserved exactly), binarize+subtract on device
(DVE), then a K=4096 accumulated matmul with x tiles stationary and ternary
weights moving. Output fp32.
"""

import numpy as np
import ml_dtypes

P = 128
N_TOK, IN_F, OUT_F = 8192, 4096, 4096
K_SUB = IN_F // P             # 32 k-subtiles
N_FREE = 512                  # matmul moving free dim (one PSUM bank of fp32)

# default sharding: token shards x out shards = 8 cores
TA, OB = 2, 4
T_TILE = 256

_CACHE = {}


def _build(repeats=1, ta=TA, ob=OB, t_tile=T_TILE, wbufs=10, psum_bufs=None):
    key = ("nc", repeats, ta, ob, t_tile, wbufs, psum_bufs)
    if key in _CACHE:
        return _CACHE[key]
    import concourse.bacc as bacc
    import concourse.mybir as mybir
    import concourse.tile as tile

    t_s = N_TOK // ta             # tokens per shard
    o_s = OUT_F // ob             # out features per shard
    n_tt = t_s // t_tile
    m_sub = t_tile // P
    if psum_bufs is None:
        # psum tile is [P, o_s] fp32 = o_s/512 banks; use all 8 banks
        psum_bufs = max(2, 8 // (o_s // N_FREE))

    nc = bacc.Bacc("TRN2", target_bir_lowering=False, debug=False)
    xP = nc.dram_tensor("xP", (n_tt, P, K_SUB, t_tile), mybir.dt.bfloat16,
                        kind="ExternalInput")
    wpT = nc.dram_tensor("wpT", (IN_F, o_s), mybir.dt.bfloat16,
                         kind="ExternalInput")
    wnT = nc.dram_tensor("wnT", (IN_F, o_s), mybir.dt.bfloat16,
                         kind="ExternalInput")
    y = nc.dram_tensor("y", (t_s, o_s), mybir.dt.float32, kind="ExternalOutput")

    wpT_r = wpT[:].rearrange("(ko ki) o -> ki ko o", ki=P)   # [128, 32, o_s]
    wnT_r = wnT[:].rearrange("(ko ki) o -> ki ko o", ki=P)
    y_r = y[:].rearrange("(to ti) o -> ti to o", ti=P)       # [128, t_s/128, o_s]

    with tile.TileContext(nc) as tc:
        with (
            tc.tile_pool(name="tern", bufs=1) as tern_pool,
            tc.tile_pool(name="wstage", bufs=wbufs) as wstage,
            tc.tile_pool(name="btmp", bufs=4) as btmp,
            tc.tile_pool(name="xp", bufs=3) as xp,
            tc.tile_pool(name="outp", bufs=3) as outp,
            tc.tile_pool(name="psum", bufs=psum_bufs, space="PSUM") as psum_pool,
        ):
            for _rep in range(repeats):
                # ---- Phase A: ternary weights, K-major, SBUF-resident ----
                ternT = tern_pool.tile([P, K_SUB, o_s], mybir.dt.bfloat16)
                for k in range(K_SUB):
                    wp_t = wstage.tile([P, o_s], mybir.dt.bfloat16, tag="w")
                    wn_t = wstage.tile([P, o_s], mybir.dt.bfloat16, tag="w")
                    nc.sync.dma_start(wp_t[:], wpT_r[:, k, :])
                    nc.scalar.dma_start(wn_t[:], wnT_r[:, k, :])
                    bn = btmp.tile([P, o_s], mybir.dt.bfloat16, tag="b")
                    nc.vector.tensor_scalar(
                        bn[:], wn_t[:], 0.0, None, mybir.AluOpType.is_gt
                    )
                    # ternT = (wp > 0) - (wn > 0)
                    nc.vector.scalar_tensor_tensor(
                        ternT[:, k, :], wp_t[:], 0.0, bn[:],
                        mybir.AluOpType.is_gt, mybir.AluOpType.subtract,
                    )

                # ---- Phase B: y[t, o] = sum_k xT[k, t] * ternT[k, o] ----
                for tt in range(n_tt):
                    x_t = xp.tile([P, K_SUB, t_tile], mybir.dt.bfloat16)
                    for kc in range(0, K_SUB, 8):
                        nc.gpsimd.dma_start(
                            x_t[:, kc:kc + 8, :], xP[tt, :, kc:kc + 8, :])
                    for m in range(m_sub):
                        ps = psum_pool.tile([P, o_s], mybir.dt.float32)
                        for k in range(K_SUB):
                            for ob2 in range(o_s // N_FREE):
                                nc.tensor.matmul(
                                    ps[:, ob2 * N_FREE:(ob2 + 1) * N_FREE],
                                    x_t[:, k, m * P:(m + 1) * P],
                                    ternT[:, k,
                                          ob2 * N_FREE:(ob2 + 1) * N_FREE],
                                    start=(k == 0),
                                    stop=(k == K_SUB - 1),
                                )
                        o_t = outp.tile([P, o_s], mybir.dt.float32)
                        nc.vector.tensor_copy(o_t[:], ps[:])
                        nc.gpsimd.dma_start(y_r[:, tt * m_sub + m, :], o_t[:])
    nc.compile()
    _CACHE[key] = nc
    return nc


def _shard_inputs(x, w_pos, w_neg, ta=TA, ob=OB, t_tile=T_TILE):
    bf16 = ml_dtypes.bfloat16
    t_s = N_TOK // ta
    o_s = OUT_F // ob
    n_tt = t_s // t_tile
    xb = x.astype(bf16)                               # [N_TOK, IN_F]
    wpT = np.ascontiguousarray(w_pos.astype(bf16).T)  # [IN_F, OUT_F]
    wnT = np.ascontiguousarray(w_neg.astype(bf16).T)
    in_maps = []
    for c in range(8):
        tai, obi = divmod(c, ob)
        xs = xb[tai * t_s:(tai + 1) * t_s]            # [t_s, IN_F]
        # [tt, t, ko, ki] -> [tt, ki, ko, t]
        xp = np.ascontiguousarray(
            xs.reshape(n_tt, t_tile, K_SUB, P).transpose(0, 3, 2, 1))
        in_maps.append({
            "xP": xp,
            "wpT": np.ascontiguousarray(wpT[:, obi * o_s:(obi + 1) * o_s]),
            "wnT": np.ascontiguousarray(wnT[:, obi * o_s:(obi + 1) * o_s]),
        })
    return in_maps


def _gather(results, ta=TA, ob=OB):
    t_s = N_TOK // ta
    o_s = OUT_F // ob
    y_full = np.empty((N_TOK, OUT_F), np.float32)
    for c in range(8):
        tai, obi = divmod(c, ob)
        y_full[tai * t_s:(tai + 1) * t_s,
               obi * o_s:(obi + 1) * o_s] = results[c]["y"]
    return y_full


def run(x, w_pos, w_neg, trace=False):
    """Returns (y_full, BassKernelResults)."""
    from concourse import bass_utils

    nc = _build()
    in_maps = _shard_inputs(x, w_pos, w_neg)
    res = bass_utils.run_bass_kernel_spmd(
        nc, in_maps, core_ids=list(range(8)), trace=trace
    )
    return _gather(res.results), res


def kernel(x, w_pos, w_neg):
    y, _ = run(x, w_pos, w_neg, trace=False)
    return y


# revision 22
# speedup vs baseline: 1.3150x; 1.0471x over previous
"""Trainium2 Bass kernel for ternary-weight linear (plinear STE forward).

Reference math:
    y = x @ ((w_pos > 0) - (w_neg > 0)).T      # [8192, 4096]

Algebraic fold: the two binarized matmuls collapse into ONE matmul with a
ternary {-1,0,1} weight matrix, halving PE work. Ternary values are exact in
bf16, so the matmul runs at bf16 rate (2x fp32); only x is quantized to bf16.

Sharding (8 cores): TA token-shards x OB out-feature shards.
Per core: x pre-tiled on host (so K=in_features lands on SBUF partitions and
every DMA is >=4KB-contiguous per partition), w slices staged transposed
(bf16 cast only -- sign is preserved exactly), binarize+subtract on device
(DVE), then a K=4096 accumulated matmul with x tiles stationary and ternary
weights moving. Output fp32.
"""

import numpy as np
import ml_dtypes

P = 128
N_TOK, IN_F, OUT_F = 8192, 4096, 4096
K_SUB = IN_F // P             # 32 k-subtiles
N_FREE = 512                  # matmul moving free dim (one PSUM bank of fp32)

# default sharding: token shards x out shards = 8 cores
TA, OB = 2, 4
T_TILE = 256

_CACHE = {}


def _enable_ldw_opt():
    """Flip walrus's --enable-ldw-opt to true (dedupes redundant LDWEIGHTS
    emitted for consecutive matmuls that share a stationary operand)."""
    if _CACHE.get("ldw_patched"):
        return
    from concourse import bass_utils as bu
    orig = bu.run_command

    def patched(argv, **kw):
        argv = ["--enable-ldw-opt=true" if a == "--enable-ldw-opt=false" else a
                for a in argv]
        return orig(argv, **kw)

    bu.run_command = patched
    _CACHE["ldw_patched"] = True


def _mm_noweights(nc, mybir, out, rhs, start, stop):
    """Raw InstMatmult with no weights operand: reuses the PE array's
    already-loaded stationary weights (walrus emits no LDWEIGHTS)."""
    te = nc.tensor
    ifmap_ap = te.lower_ap(rhs.opt({0}), opt=False)
    out_ap = te.lower_ap(out)
    return te.add_instruction(
        mybir.InstMatmult(
            name=nc.get_next_instruction_name(),
            replication_resolution=0,
            replication_shift_amnt=0,
            replication_num_rows=0,
            start_tensor_calc=start,
            stop_tensor_calc=stop,
            ins=[ifmap_ap],
            outs=[out_ap],
            perf_mode=None,
            is_transpose=None,
            ifmap_quant_offset=None,
            weights_quant_offset=None,
            bass_skip_group_check=False,
            tile_position=(0, 0),
            tile_size=(128, 128),
        )
    )


def _build(repeats=1, ta=TA, ob=OB, t_tile=T_TILE, wbufs=10, psum_bufs=None,
           xbufs=3, obufs=3, bbufs=4, dedup_ldw=False):
    key = ("nc", repeats, ta, ob, t_tile, wbufs, psum_bufs, xbufs, obufs,
           bbufs, dedup_ldw)
    if key in _CACHE:
        return _CACHE[key]
    import concourse.bacc as bacc
    import concourse.mybir as mybir
    import concourse.tile as tile

    t_s = N_TOK // ta             # tokens per shard
    o_s = OUT_F // ob             # out features per shard
    n_tt = t_s // t_tile
    m_sub = t_tile // P
    if psum_bufs is None:
        # psum tile is [P, o_s] fp32 = o_s/512 banks; use all 8 banks
        psum_bufs = max(2, 8 // (o_s // N_FREE))

    nc = bacc.Bacc("TRN2", target_bir_lowering=False, debug=False)
    xP = nc.dram_tensor("xP", (n_tt, P, K_SUB, t_tile), mybir.dt.bfloat16,
                        kind="ExternalInput")
    wpT = nc.dram_tensor("wpT", (IN_F, o_s), mybir.dt.bfloat16,
                         kind="ExternalInput")
    wnT = nc.dram_tensor("wnT", (IN_F, o_s), mybir.dt.bfloat16,
                         kind="ExternalInput")
    y = nc.dram_tensor("y", (t_s, o_s), mybir.dt.float32, kind="ExternalOutput")

    wpT_r = wpT[:].rearrange("(ko ki) o -> ki ko o", ki=P)   # [128, 32, o_s]
    wnT_r = wnT[:].rearrange("(ko ki) o -> ki ko o", ki=P)
    y_r = y[:].rearrange("(to ti) o -> ti to o", ti=P)       # [128, t_s/128, o_s]

    with tile.TileContext(nc) as tc:
        with (
            tc.tile_pool(name="tern", bufs=1) as tern_pool,
            tc.tile_pool(name="wstage", bufs=wbufs) as wstage,
            tc.tile_pool(name="btmp", bufs=bbufs) as btmp,
            tc.tile_pool(name="xp", bufs=xbufs) as xp,
            tc.tile_pool(name="outp", bufs=obufs) as outp,
            tc.tile_pool(name="psum", bufs=psum_bufs, space="PSUM") as psum_pool,
        ):
            for _rep in range(repeats):
                # ---- Phase A: ternary weights, K-major, SBUF-resident ----
                ternT = tern_pool.tile([P, K_SUB, o_s], mybir.dt.bfloat16)
                for k in range(K_SUB):
                    wp_t = wstage.tile([P, o_s], mybir.dt.bfloat16, tag="w")
                    wn_t = wstage.tile([P, o_s], mybir.dt.bfloat16, tag="w")
                    h = o_s // 2
                    nc.sync.dma_start(wp_t[:, :h], wpT_r[:, k, :h])
                    nc.scalar.dma_start(wp_t[:, h:], wpT_r[:, k, h:])
                    nc.scalar.dma_start(wn_t[:, :h], wnT_r[:, k, :h])
                    nc.sync.dma_start(wn_t[:, h:], wnT_r[:, k, h:])
                    bn = btmp.tile([P, o_s], mybir.dt.bfloat16, tag="b")
                    nc.vector.tensor_scalar(
                        bn[:], wn_t[:], 0.0, None, mybir.AluOpType.is_gt
                    )
                    # ternT = (wp > 0) - (wn > 0)
                    nc.vector.scalar_tensor_tensor(
                        ternT[:, k, :], wp_t[:], 0.0, bn[:],
                        mybir.AluOpType.is_gt, mybir.AluOpType.subtract,
                    )

                # ---- Phase B: y[t, o] = sum_k xT[k, t] * ternT[k, o] ----
                for tt in range(n_tt):
                    x_t = xp.tile([P, K_SUB, t_tile], mybir.dt.bfloat16)
                    for ci, kc in enumerate(range(0, K_SUB, 8)):
                        eng = nc.sync if ci % 2 == 0 else nc.scalar
                        eng.dma_start(
                            x_t[:, kc:kc + 8, :], xP[tt, :, kc:kc + 8, :])
                    for m in range(m_sub):
                        ps = psum_pool.tile([P, o_s], mybir.dt.float32)
                        for k in range(K_SUB):
                            for ob2 in range(o_s // N_FREE):
                                out_sl = ps[:, ob2 * N_FREE:(ob2 + 1) * N_FREE]
                                rhs_sl = ternT[:, k,
                                               ob2 * N_FREE:(ob2 + 1) * N_FREE]
                                if dedup_ldw and ob2 > 0:
                                    _mm_noweights(
                                        nc, mybir, out_sl, rhs_sl,
                                        start=(k == 0), stop=(k == K_SUB - 1))
                                else:
                                    nc.tensor.matmul(
                                        out_sl,
                                        x_t[:, k, m * P:(m + 1) * P],
                                        rhs_sl,
                                        start=(k == 0),
                                        stop=(k == K_SUB - 1),
                                    )
                        o_t = outp.tile([P, o_s], mybir.dt.float32)
                        nc.vector.tensor_copy(o_t[:], ps[:])
                        eng = nc.sync if (tt * m_sub + m) % 2 == 0 else nc.scalar
                        eng.dma_start(y_r[:, tt * m_sub + m, :], o_t[:])
    nc.compile()
    _CACHE[key] = nc
    return nc


def _shard_inputs(x, w_pos, w_neg, ta=TA, ob=OB, t_tile=T_TILE):
    bf16 = ml_dtypes.bfloat16
    t_s = N_TOK // ta
    o_s = OUT_F // ob
    n_tt = t_s // t_tile
    xb = x.astype(bf16)                               # [N_TOK, IN_F]
    wpT = np.ascontiguousarray(w_pos.astype(bf16).T)  # [IN_F, OUT_F]
    wnT = np.ascontiguousarray(w_neg.astype(bf16).T)
    in_maps = []
    for c in range(8):
        tai, obi = divmod(c, ob)
        xs = xb[tai * t_s:(tai + 1) * t_s]            # [t_s, IN_F]
        # [tt, t, ko, ki] -> [tt, ki, ko, t]
        xp = np.ascontiguousarray(
            xs.reshape(n_tt, t_tile, K_SUB, P).transpose(0, 3, 2, 1))
        in_maps.append({
            "xP": xp,
            "wpT": np.ascontiguousarray(wpT[:, obi * o_s:(obi + 1) * o_s]),
            "wnT": np.ascontiguousarray(wnT[:, obi * o_s:(obi + 1) * o_s]),
        })
    return in_maps


def _gather(results, ta=TA, ob=OB):
    t_s = N_TOK // ta
    o_s = OUT_F // ob
    y_full = np.empty((N_TOK, OUT_F), np.float32)
    for c in range(8):
        tai, obi = divmod(c, ob)
        y_full[tai * t_s:(tai + 1) * t_s,
               obi * o_s:(obi + 1) * o_s] = results[c]["y"]
    return y_full


def run(x, w_pos, w_neg, trace=False):
    """Returns (y_full, BassKernelResults)."""
    from concourse import bass_utils

    nc = _build()
    in_maps = _shard_inputs(x, w_pos, w_neg)
    res = bass_utils.run_bass_kernel_spmd(
        nc, in_maps, core_ids=list(range(8)), trace=trace
    )
    return _gather(res.results), res


def kernel(x, w_pos, w_neg):
    y, _ = run(x, w_pos, w_neg, trace=False)
    return y


# revision 28
# speedup vs baseline: 1.4882x; 1.1317x over previous
"""Trainium2 Bass kernel for ternary-weight linear (plinear STE forward).

Reference math:
    y = x @ ((w_pos > 0) - (w_neg > 0)).T      # [8192, 4096]

Algebraic fold: the two binarized matmuls collapse into ONE matmul with a
ternary {-1,0,1} weight matrix, halving PE work. Ternary values are exact in
bf16, so the matmul runs at bf16 rate (2x fp32); only x is quantized to bf16.

Sharding (8 cores): TA token-shards x OB out-feature shards.
Per core: x pre-tiled on host (so K=in_features lands on SBUF partitions and
every DMA is >=4KB-contiguous per partition), w slices staged transposed
(bf16 cast only -- sign is preserved exactly), binarize+subtract on device
(DVE), then a K=4096 accumulated matmul with x tiles stationary and ternary
weights moving. Output fp32.
"""

import numpy as np
import ml_dtypes

P = 128
N_TOK, IN_F, OUT_F = 8192, 4096, 4096
K_SUB = IN_F // P             # 32 k-subtiles
N_FREE = 512                  # matmul moving free dim (one PSUM bank of fp32)

# default sharding: token shards x out shards = 8 cores.
# ta=4/ob=2 halves the count of distinct PE stationary operands (weight
# changes) vs ta=2/ob=4 -- measured ~150ns pipeline bubble per change on HW.
TA, OB = 4, 2
T_TILE = 128

_CACHE = {}


def _enable_ldw_opt():
    """Flip walrus's --enable-ldw-opt to true (dedupes redundant LDWEIGHTS
    emitted for consecutive matmuls that share a stationary operand)."""
    if _CACHE.get("ldw_patched"):
        return
    from concourse import bass_utils as bu
    orig = bu.run_command

    def patched(argv, **kw):
        argv = ["--enable-ldw-opt=true" if a == "--enable-ldw-opt=false" else a
                for a in argv]
        return orig(argv, **kw)

    bu.run_command = patched
    _CACHE["ldw_patched"] = True


def _mm_noweights(nc, mybir, out, rhs, start, stop):
    """Raw InstMatmult with no weights operand: reuses the PE array's
    already-loaded stationary weights (walrus emits no LDWEIGHTS)."""
    te = nc.tensor
    ifmap_ap = te.lower_ap(rhs.opt({0}), opt=False)
    out_ap = te.lower_ap(out)
    return te.add_instruction(
        mybir.InstMatmult(
            name=nc.get_next_instruction_name(),
            replication_resolution=0,
            replication_shift_amnt=0,
            replication_num_rows=0,
            start_tensor_calc=start,
            stop_tensor_calc=stop,
            ins=[ifmap_ap],
            outs=[out_ap],
            perf_mode=None,
            is_transpose=None,
            ifmap_quant_offset=None,
            weights_quant_offset=None,
            bass_skip_group_check=False,
            tile_position=(0, 0),
            tile_size=(128, 128),
        )
    )


def _build(repeats=1, ta=TA, ob=OB, t_tile=T_TILE, wbufs=4, psum_bufs=None,
           xbufs=2, obufs=2, bbufs=2, dedup_ldw=False):
    key = ("nc", repeats, ta, ob, t_tile, wbufs, psum_bufs, xbufs, obufs,
           bbufs, dedup_ldw)
    if key in _CACHE:
        return _CACHE[key]
    import concourse.bacc as bacc
    import concourse.mybir as mybir
    import concourse.tile as tile

    t_s = N_TOK // ta             # tokens per shard
    o_s = OUT_F // ob             # out features per shard
    n_tt = t_s // t_tile
    m_sub = t_tile // P
    if psum_bufs is None:
        # psum tile is [P, o_s] fp32 = o_s/512 banks; use all 8 banks
        psum_bufs = max(2, 8 // (o_s // N_FREE))

    nc = bacc.Bacc("TRN2", target_bir_lowering=False, debug=False)
    # x and w both pre-tiled on host so every DMA is a large per-partition
    # contiguous block.
    xP = nc.dram_tensor("xP", (n_tt, P, K_SUB, t_tile), mybir.dt.bfloat16,
                        kind="ExternalInput")
    wpQ = nc.dram_tensor("wpQ", (P, K_SUB, o_s), mybir.dt.bfloat16,
                         kind="ExternalInput")
    wnQ = nc.dram_tensor("wnQ", (P, K_SUB, o_s), mybir.dt.bfloat16,
                         kind="ExternalInput")
    y = nc.dram_tensor("y", (t_s, o_s), mybir.dt.float32, kind="ExternalOutput")

    y_r = y[:].rearrange("(to ti) o -> ti to o", ti=P)       # [128, t_s/128, o_s]

    with tile.TileContext(nc) as tc:
        with (
            tc.tile_pool(name="tern", bufs=1) as tern_pool,
            tc.tile_pool(name="wstage", bufs=wbufs) as wstage,
            tc.tile_pool(name="btmp", bufs=bbufs) as btmp,
            tc.tile_pool(name="xp", bufs=xbufs) as xp,
            tc.tile_pool(name="outp", bufs=obufs) as outp,
            tc.tile_pool(name="psum", bufs=psum_bufs, space="PSUM") as psum_pool,
        ):
            for _rep in range(repeats):
                # ---- Phase A: ternary weights, K-major, SBUF-resident ----
                KG = 2  # k-slices per w load group
                ternT = tern_pool.tile([P, K_SUB, o_s], mybir.dt.bfloat16)
                for k0 in range(0, K_SUB, KG):
                    wp_t = wstage.tile([P, KG, o_s], mybir.dt.bfloat16, tag="w")
                    wn_t = wstage.tile([P, KG, o_s], mybir.dt.bfloat16, tag="w")
                    nc.sync.dma_start(wp_t[:], wpQ[:, k0:k0 + KG, :])
                    nc.scalar.dma_start(wn_t[:], wnQ[:, k0:k0 + KG, :])
                    for j in range(KG):
                        bn = btmp.tile([P, o_s], mybir.dt.bfloat16, tag="b")
                        nc.vector.tensor_scalar(
                            bn[:], wn_t[:, j, :], 0.0, None,
                            mybir.AluOpType.is_gt
                        )
                        # ternT = (wp > 0) - (wn > 0)
                        nc.vector.scalar_tensor_tensor(
                            ternT[:, k0 + j, :], wp_t[:, j, :], 0.0, bn[:],
                            mybir.AluOpType.is_gt, mybir.AluOpType.subtract,
                        )

                # ---- Phase B: y[t, o] = sum_k xT[k, t] * ternT[k, o] ----
                for tt in range(n_tt):
                    x_t = xp.tile([P, K_SUB, t_tile], mybir.dt.bfloat16)
                    if tt == 0:
                        # chunked so the first matmuls can start early
                        for ci, kc in enumerate(range(0, K_SUB, 8)):
                            eng = nc.sync if ci % 2 == 0 else nc.scalar
                            eng.dma_start(
                                x_t[:, kc:kc + 8, :], xP[tt, :, kc:kc + 8, :])
                    else:
                        eng = nc.sync if tt % 2 == 0 else nc.scalar
                        eng.dma_start(x_t[:], xP[tt])
                    for m in range(m_sub):
                        ps = psum_pool.tile([P, o_s], mybir.dt.float32)
                        for k in range(K_SUB):
                            for ob2 in range(o_s // N_FREE):
                                out_sl = ps[:, ob2 * N_FREE:(ob2 + 1) * N_FREE]
                                rhs_sl = ternT[:, k,
                                               ob2 * N_FREE:(ob2 + 1) * N_FREE]
                                if dedup_ldw and ob2 > 0:
                                    _mm_noweights(
                                        nc, mybir, out_sl, rhs_sl,
                                        start=(k == 0), stop=(k == K_SUB - 1))
                                else:
                                    nc.tensor.matmul(
                                        out_sl,
                                        x_t[:, k, m * P:(m + 1) * P],
                                        rhs_sl,
                                        start=(k == 0),
                                        stop=(k == K_SUB - 1),
                                    )
                        o_t = outp.tile([P, o_s], mybir.dt.float32)
                        nc.vector.tensor_copy(o_t[:], ps[:])
                        eng = nc.sync if (tt * m_sub + m) % 2 == 0 else nc.scalar
                        eng.dma_start(y_r[:, tt * m_sub + m, :], o_t[:])
    nc.compile()
    _CACHE[key] = nc
    return nc


def _shard_inputs(x, w_pos, w_neg, ta=TA, ob=OB, t_tile=T_TILE):
    bf16 = ml_dtypes.bfloat16
    t_s = N_TOK // ta
    o_s = OUT_F // ob
    n_tt = t_s // t_tile
    xb = x.astype(bf16)                               # [N_TOK, IN_F]
    wpT = np.ascontiguousarray(w_pos.astype(bf16).T)  # [IN_F, OUT_F]
    wnT = np.ascontiguousarray(w_neg.astype(bf16).T)
    in_maps = []
    for c in range(8):
        tai, obi = divmod(c, ob)
        xs = xb[tai * t_s:(tai + 1) * t_s]            # [t_s, IN_F]
        # [tt, t, ko, ki] -> [tt, ki, ko, t]
        xp = np.ascontiguousarray(
            xs.reshape(n_tt, t_tile, K_SUB, P).transpose(0, 3, 2, 1))
        # w: [in=(ko ki), o] -> [ki, ko, o] so k-groups are contiguous
        wq_p = np.ascontiguousarray(
            wpT[:, obi * o_s:(obi + 1) * o_s]
            .reshape(K_SUB, P, o_s).transpose(1, 0, 2))
        wq_n = np.ascontiguousarray(
            wnT[:, obi * o_s:(obi + 1) * o_s]
            .reshape(K_SUB, P, o_s).transpose(1, 0, 2))
        in_maps.append({"xP": xp, "wpQ": wq_p, "wnQ": wq_n})
    return in_maps


def _gather(results, ta=TA, ob=OB):
    t_s = N_TOK // ta
    o_s = OUT_F // ob
    y_full = np.empty((N_TOK, OUT_F), np.float32)
    for c in range(8):
        tai, obi = divmod(c, ob)
        y_full[tai * t_s:(tai + 1) * t_s,
               obi * o_s:(obi + 1) * o_s] = results[c]["y"]
    return y_full


def run(x, w_pos, w_neg, trace=False):
    """Returns (y_full, BassKernelResults)."""
    from concourse import bass_utils

    nc = _build()
    in_maps = _shard_inputs(x, w_pos, w_neg)
    res = bass_utils.run_bass_kernel_spmd(
        nc, in_maps, core_ids=list(range(8)), trace=trace
    )
    return _gather(res.results), res


def kernel(x, w_pos, w_neg):
    y, _ = run(x, w_pos, w_neg, trace=False)
    return y
